# revision 1
# baseline (speedup 1.0000x reference)
"""FUSE bucket-model scan kernel for Trainium2 (8 NeuronCores).

Strategy
--------
H=4096 HRUs are sharded across 8 cores (512 each). Each core holds its HRUs
as [128 partitions x 4 groups]; the two bucket states are packed with the
groups into [128, 8] tiles (cols 0-3: upper zone per group, cols 4-7: lower
zone per group). The T=8192 time recurrence runs as a sequential scan on
the device; all forcing-derived per-step operand tiles (Z = [p*inv1 |
baserte*inv2], A = [-(pet+percrte)*inv1 | percrte*inv2]) are precomputed on
the host in a scan-friendly [128, T, 8] layout and streamed in per K-step
chunk. The loop-carried chain per step (everything else runs in its shadow):

  r -> l = Ln(r) -> m = l*PW -> x = Exp(m) -> h = x*Z_t
    -> r' = clip(phi - h, eps, 1)   [custom fused DVE op]

with off-chain per step:
  w_l/w_r = r1 * A_t; phi = sigma + w (+pn on the left half)
  sigma' = clip(phi - h, 0, 1)       [custom fused DVE op]
  runoff = (h*MW) pair-summed into the chunk output tile

State is normalized (sigma = s/maxwatr in [0,1]) so both clips take
immediate bounds and fuse into single instructions. The scan is
latency-bound (~1.6us/step: two ACT transcendental round-trips + three
DVE nodes); streaming, runoff output, and the phi/w arithmetic all fit
inside that latency shadow (h/m and runoff ops on Pool keep the DVE
queue clear for the fused clips). Model-estimated device time ~12.3 ms;
output matches the jax reference to ~8e-6 relative-of-max.
"""
import numpy as np

import concourse.bass as bass
import concourse.bacc as bacc
import concourse.mybir as mybir
from concourse.bass import ds
from concourse.tile import TileContext
from concourse.bass_utils import run_bass_kernel_spmd

F32 = mybir.dt.float32
AF = mybir.ActivationFunctionType
OP = mybir.AluOpType


# --- custom fused DVE ops ---------------------------------------------------
# The loop-carried chain is sigma/r -> Ln -> mul -> Exp -> h -> next state.
# Fusing "u = phi - h; sigma' = clip(u,0,1)" and "r' = clip(u,eps,1)" into one
# custom DVE instruction each removes two DVE nodes from that chain (r' feeds
# the next Ln directly; sigma' is consumed off-chain by the phi ops).
from concourse.dve_spec import Spec, Src0, Src1, maxx, minn, lower as _dve_lower
from concourse.dve_spec import C0 as _C0, One as _One, Zero as _Zero
from concourse import dve_ops as _dvo
from concourse.dve_uop import DveOpSpec as _DveOpSpec


def _register_custom_op(name, spec):
    for op in _dvo.OPS:
        if op.name == name:
            return op
    row = _dvo._CUSTOM_DVE_ROW_BASE + len(_dvo.OPS)
    _dvo._SUB_OPCODE_FOR_NAME[name] = row
    shas = {}
    for ver in ("v3", "v4"):
        try:
            uops = _dve_lower(spec, ver=ver)
            shas[ver] = _DveOpSpec(name=name, opcode=row, uops=uops,
                                   rd1_en=True).sha(ver)
        except Exception:
            pass
    op = _dvo.DveOp(name, spec, subdim=False, uops_sha=shas)
    _dvo.OPS.append(op)
    _dvo.CUSTOM_DVE_SPECS[name] = spec
    return op


SIG_CLIPSUB = _register_custom_op(
    "FUSE_SIG_CLIPSUB",
    Spec(
        body=minn(maxx(Src0 - Src1, _Zero), _One),
        reference=lambda in0, in1, s0, s1, imm2: np.clip(
            (in0.astype(np.float32) - in1).astype(np.float32),
            np.float32(0.0), np.float32(1.0)),
    ),
)
R_CLIPSUB = _register_custom_op(
    "FUSE_R_CLIPSUB",
    Spec(
        body=minn(maxx(Src0 - Src1, _C0), _One),
        reference=lambda in0, in1, s0, s1, imm2: np.clip(
            (in0.astype(np.float32) - in1).astype(np.float32),
            np.float32(s0), np.float32(1.0)),
    ),
)

T = 8192
H = 4096
NCORES = 8
HC = H // NCORES          # 512 HRUs per core
P = 128                   # partitions
G = HC // P               # 4 groups
K = 128                   # timesteps per chunk
EPS = 1e-6


def build_nc(t_total=T, k_chunk=K, unrolled=False, ro_pool=True, w_pool=False, m_pool=True, w3d=True, prio=False, h_pool=True, zc2_pool=False, wbufs=4):
    nc = bacc.Bacc()
    ZAt = nc.dram_tensor("ZA", [P, t_total * 16], F32, kind="ExternalInput")
    Ct = nc.dram_tensor("CONSTS", [P, 24], F32, kind="ExternalInput")
    RO = nc.dram_tensor("RO", [G * P, t_total], F32, kind="ExternalOutput")

    n_chunks = t_total // k_chunk
    with TileContext(nc) as tc:
        with (
            tc.tile_pool(name="const", bufs=1) as cpool,
            tc.tile_pool(name="zin", bufs=3) as zpool,
            tc.tile_pool(name="rout", bufs=3) as ropool,
            tc.tile_pool(name="work", bufs=wbufs) as wpool,
        ):
            cst_in = cpool.tile([P, 24], F32)
            cst = cpool.tile([P, 24], F32)
            nc.sync.dma_start(out=cst_in[:], in_=Ct[:])
            # pre-loop copy: the loop body then only depends on DVE-written
            # tiles for its constants, keeping per-instruction wait lists small
            nc.vector.tensor_copy(out=cst[:], in_=cst_in[:])
            pw = cst[:, 8:16]
            mw = cst[:, 16:24]
            sigt = cpool.tile([P, 8], F32)
            rrt = cpool.tile([P, 8], F32)
            nc.vector.tensor_copy(out=sigt[:], in_=cst[:, 0:8])
            nc.vector.tensor_scalar(out=rrt[:], in0=cst[:, 0:8], scalar1=EPS,
                                    scalar2=None, op0=OP.max)
            sig = sigt[:]

            import contextlib
            def chunk_iter():
                if unrolled:
                    for i in range(n_chunks):
                        yield contextlib.nullcontext(i)
                else:
                    yield tc.For_i(0, n_chunks, staggered_reset=True,
                                   hint_engines=(mybir.EngineType.DVE,
                                                 mybir.EngineType.Activation))
            for _cm in chunk_iter():
              with _cm as ci:
                  zc = zpool.tile([P, k_chunk * 16], F32)
                  ro = ropool.tile([P, k_chunk * 4], F32)
                  nc.sync.dma_start(out=zc[:], in_=ZAt[:, ds(ci * (k_chunk * 16), k_chunk * 16)])
                  # DVE pre-touch: the DVE ISA struct allows only one semaphore
                  # wait per instruction, so absorb the DMA-completion wait in a
                  # dedicated copy; the scan ops then only wait on ACT.
                  zc2 = zpool.tile([P, k_chunk * 16], F32, tag="zc2")
                  (nc.gpsimd if zc2_pool else nc.vector).tensor_copy(out=zc2[:], in_=zc[:])

                  for k in range(k_chunk):
                      z8 = zc2[:, k * 16:k * 16 + 8]
                      a8 = zc2[:, k * 16 + 8:k * 16 + 16]
                      l = wpool.tile([P, 8], F32, tag="l")
                      m = wpool.tile([P, 8], F32, tag="m")
                      x = wpool.tile([P, 8], F32, tag="x")
                      h = wpool.tile([P, 8], F32, tag="h")
                      w = wpool.tile([P, 8], F32, tag="w")
                      phi = wpool.tile([P, 8], F32, tag="phi")
                      ra = wpool.tile([P, 8], F32, tag="ra")

                      i1 = nc.scalar.activation(l[:], rrt[:], AF.Ln)
                      meng = nc.gpsimd if m_pool else nc.vector
                      i2 = meng.tensor_tensor(out=m[:], in0=l[:], in1=pw, op=OP.mult)
                      i3 = nc.scalar.activation(x[:], m[:], AF.Exp)
                      heng = nc.gpsimd if h_pool else nc.vector
                      i4 = heng.tensor_tensor(out=h[:], in0=x[:], in1=z8, op=OP.mult)
                      if prio:
                          for bi in (i1, i2, i3, i4):
                              bi.ins.bass_priority = -100

                      weng = nc.gpsimd if w_pool else nc.vector
                      if w3d:
                          r1b = rrt[:, 0:4].rearrange('p (o f) -> p o f', o=1) \
                                           .broadcast_to([P, 2, 4])
                          a3 = a8.rearrange('p (o f) -> p o f', o=2)
                          w3 = w[:].rearrange('p (o f) -> p o f', o=2)
                          weng.tensor_tensor(out=w3, in0=r1b, in1=a3, op=OP.mult)
                      else:
                          weng.tensor_tensor(out=w[:, 0:4], in0=rrt[:, 0:4],
                                                  in1=a8[:, 0:4], op=OP.mult)
                          weng.tensor_tensor(out=w[:, 4:8], in0=rrt[:, 0:4],
                                                  in1=a8[:, 4:8], op=OP.mult)
                      weng.tensor_tensor(out=phi[:], in0=sig, in1=w[:], op=OP.add)
                      weng.tensor_tensor(out=phi[:, 0:4], in0=phi[:, 0:4],
                                              in1=z8[:, 0:4], op=OP.add)
                      # fused (phi - h) -> clip: state and next-r in one node each
                      nc.vector._custom_dve(SIG_CLIPSUB, out=sig, in0=phi[:], in1=h[:])
                      nc.vector._custom_dve(R_CLIPSUB, out=rrt[:], in0=phi[:],
                                            in1=h[:], s0=EPS)

                      roeng = nc.gpsimd if ro_pool else nc.vector
                      roeng.tensor_tensor(out=ra[:], in0=h[:], in1=mw, op=OP.mult)
                      rocol = ro[:].rearrange('p (g t) -> p g t', g=G)[:, :, k]
                      roeng.tensor_tensor(out=rocol,
                                              in0=ra[:, 0:4], in1=ra[:, 4:8], op=OP.add)

                  rov = ro[:].rearrange('p (g t) -> p g t', g=G)
                  dst = RO.rearrange('(g p) t -> p g t', g=G)[:, :, ds(ci * k_chunk, k_chunk)]
                  nc.sync.dma_start(out=dst, in_=rov)
    _compile_with_combined_ln_exp_table(nc)
    return nc


def _compile_with_combined_ln_exp_table(nc):
    """Bacc's act-table chooser picks separate `exp` and `ln` sets, inserting
    a ~1.3us table load before every activation (2.6us/step!). Both live in
    the `natural_log_exp_and_others` set; strip them from all other sets
    (keeping dict order, since the set id is positional) so the fixpoint
    resolves both to the combined set and hoists one load to the entry."""
    orig = bacc.get_activation_tables
    want = {mybir.ActivationFunctionType.Ln, mybir.ActivationFunctionType.Exp}

    def patched(arch):
        tabs = orig(arch)
        out = {}
        for name, funcs in tabs.items():
            if name != "natural_log_exp_and_others":
                funcs = funcs - want
            out[name] = funcs
        return out

    bacc.get_activation_tables = patched
    try:
        nc.compile()
    finally:
        bacc.get_activation_tables = orig


def _host_prepare(forcing, initial_state, raw_params, param_lower, param_upper,
                  t_total=T):
    """Derive per-core input arrays. All fp32, same op order as the sim."""
    f32 = np.float32
    lo = param_lower.astype(f32)
    hi = param_upper.astype(f32)
    # sigmoid in f64 then round: within 2ulp of jax.nn.sigmoid; end impact is nil
    sg = (1.0 / (1.0 + np.exp(-raw_params.astype(np.float64))))
    phys = (lo.astype(np.float64) + (hi - lo).astype(np.float64) * sg).astype(f32)
    mw1, mw2, percrte, baserte, qbp, axv = [phys[:, i].copy() for i in range(6)]
    inv1 = (f32(1.0) / mw1).astype(f32)
    inv2 = (f32(1.0) / mw2).astype(f32)

    p_r = forcing[:, :, 0].astype(f32)    # [T, H]
    pet = forcing[:, :, 1].astype(f32)

    pn = (p_r * inv1[None, :]).astype(f32)
    al = (-((pet + percrte[None, :]) * inv1[None, :])).astype(f32)
    bn = (baserte * inv2).astype(f32)
    pc12 = (percrte * inv2).astype(f32)

    s1n = (initial_state[:, 0].astype(f32) * inv1).astype(f32)
    s2n = (initial_state[:, 1].astype(f32) * inv2).astype(f32)

    in_maps = []
    for c in range(NCORES):
        sl = slice(c * HC, (c + 1) * HC)
        # [T, HC] -> [T, G, P] -> [P, T, G]
        def tg(a):
            return np.ascontiguousarray(
                a[:, sl].reshape(t_total, G, P).transpose(2, 0, 1))
        pn_c = tg(pn)          # [P, T, G]
        al_c = tg(al)
        ZA = np.empty((P, t_total, 16), f32)
        ZA[:, :, 0:4] = pn_c
        ZA[:, :, 4:8] = bn[sl].reshape(G, P).T[:, None, :]
        ZA[:, :, 8:12] = al_c
        ZA[:, :, 12:16] = pc12[sl].reshape(G, P).T[:, None, :]

        def pk(a1, a2):
            out = np.empty((P, 8), f32)
            out[:, 0:4] = a1[sl].reshape(G, P).T
            out[:, 4:8] = a2[sl].reshape(G, P).T
            return out

        consts = np.concatenate([pk(s1n, s2n), pk(axv, qbp), pk(mw1, mw2)],
                                axis=1)
        in_maps.append({
            "ZA": ZA.reshape(P, t_total * 16),
            "CONSTS": consts,
        })
    return in_maps


_NC_CACHE = {}


def kernel(forcing, initial_state, raw_params, param_lower, param_upper):
    forcing = np.asarray(forcing)
    initial_state = np.asarray(initial_state)
    raw_params = np.asarray(raw_params)
    param_lower = np.asarray(param_lower)
    param_upper = np.asarray(param_upper)
    t_total = forcing.shape[0]
    if t_total not in _NC_CACHE:
        _NC_CACHE[t_total] = build_nc(t_total=t_total)
    nc = _NC_CACHE[t_total]
    in_maps = _host_prepare(forcing, initial_state, raw_params,
                            param_lower, param_upper, t_total=t_total)
    res = run_bass_kernel_spmd(nc, in_maps, core_ids=list(range(NCORES)))
    # per-core RO: [T, G, P] with h_local = g*P + p
    out = np.empty((t_total, H), np.float32)
    for c in range(NCORES):
        ro = res.results[c]["RO"]           # [HC, T], row = g*P + p
        out[:, c * HC:(c + 1) * HC] = ro.T
    return out

